# revision 1
# baseline (speedup 1.0000x reference)
"""MultiLayerTetra TRN2 Bass kernel (8-core SPMD, data-parallel over queries).

Algorithm: the reference's per-step batched 4x4 solve collapses to an
incremental barycentric update. Per descent step, with cut pair (c0,c1) of
the current cell and barycentric weights w:
    d = w[c0] - w[c1]; choice g = [d > 0]
    abandoned a = cut slot with larger w, kept k = the other
    w[k] <- w[k] - w[a]  (= -|d|),  w[a] <- 2*w[a]  (= sum + |d|)
    pid[a] <- point_index[child, a];  cell <- 2*cell + 1 + g
Final: out = sum_j w_j * field[pid_j].  (Validated vs reference: ~1e-6 rel.)

Per-cell table row (6 f32): D = onehot(c0) - onehot(c1), p0, p1 - p0 where
p0/p1 are the pids injected when descending with choice 0/1.

Device mapping per core (16384 queries): query (p, q) at partition p, free
slot q; slot-interleaved W/PID state [128, QP*4]. Gathers use the [P,1]
per-partition indirect-DMA config (one offset per partition per
instruction), which is the HW-reliable vector-DGE shape.
"""
import functools
import numpy as np

import concourse.bass as bass
import concourse.bacc as bacc
import concourse.mybir as mybir
from concourse.tile import TileContext
from concourse.bass_utils import run_bass_kernel_spmd

DEPTH = 18
NT = 2 ** DEPTH - 1
P = 128
F = 32
N_CORES = 8
QP = 128
NCHUNK = 2
FIELD_ROWS = 100000

AL = mybir.AluOpType
AF = mybir.ActivationFunctionType


def _build_tables(child_cut, point_index):
    c = np.arange(NT)
    cut0 = child_cut[:NT, 0].astype(np.int64)
    cut1 = child_cut[:NT, 1].astype(np.int64)
    eye = np.eye(4, dtype=np.float32)
    D = eye[cut0] - eye[cut1]
    p0 = point_index[2 * c + 1, cut1].astype(np.float32)
    p1 = point_index[2 * c + 2, cut0].astype(np.float32)
    return np.concatenate([D, p0[:, None], (p1 - p0)[:, None]],
                          axis=1).astype(np.float32)


def _minv_from_root(root_xyz):
    M = np.concatenate(
        [root_xyz.T.astype(np.float64), np.ones((1, 4), np.float64)], axis=0)
    return np.linalg.inv(M).astype(np.float32)


def _build_kernel(nc, minv):
    f32 = mybir.dt.float32
    i32 = mybir.dt.int32
    i8 = mybir.dt.int8
    NQ = P * QP
    QC = QP // NCHUNK
    FSUB = max(1, QC // 32)
    QF = QC // FSUB

    xyzf = nc.dram_tensor("xyzf", [P, QP * 3], f32, kind="ExternalInput")
    tabs = nc.dram_tensor("tabs", [NT, 6], f32, kind="ExternalInput")
    field = nc.dram_tensor("field", [FIELD_ROWS, F], f32,
                           kind="ExternalInput")
    out = nc.dram_tensor("out", [NQ, F], f32, kind="ExternalOutput")
    outv = out[:].rearrange("(p q) f -> p (q f)", p=P)

    with TileContext(nc) as tc:
        with tc.tile_pool(name="state", bufs=1) as st, \
             tc.tile_pool(name="tmp", bufs=2) as tp, \
             tc.tile_pool(name="gath", bufs=2) as gp:

            xyzs = st.tile([P, QP * 3], f32, tag="xyzs")
            nc.sync.dma_start(out=xyzs[:], in_=xyzf[:])
            xyz3 = xyzs[:].rearrange("p (q c) -> p q c", c=3)

            W, PID, L = [], [], []
            for ci in range(NCHUNK):
                qlo = ci * QC
                Xv = xyz3[:, qlo:qlo + QC, 0]
                Yv = xyz3[:, qlo:qlo + QC, 1]
                Zv = xyz3[:, qlo:qlo + QC, 2]
                Wc = st.tile([P, QC * 4], f32, tag=f"W{ci}")
                W3 = Wc[:].rearrange("p (q s) -> p q s", s=4)
                for j in range(4):
                    a1 = tp.tile([P, QC], f32, tag=f"ia1_{ci}")
                    nc.scalar.activation(a1[:], Zv, AF.Copy,
                                         bias=float(minv[j, 3]),
                                         scale=float(minv[j, 2]))
                    a2 = tp.tile([P, QC], f32, tag=f"ia2_{ci}")
                    nc.vector.scalar_tensor_tensor(
                        out=a2[:], in0=Yv, scalar=float(minv[j, 1]),
                        in1=a1[:], op0=AL.mult, op1=AL.add)
                    nc.vector.scalar_tensor_tensor(
                        out=W3[:, :, j], in0=Xv, scalar=float(minv[j, 0]),
                        in1=a2[:], op0=AL.mult, op1=AL.add)
                PIDc = st.tile([P, QC * 4], f32, tag=f"PID{ci}")
                pii = tp.tile([P, QC * 4], i32, tag=f"pii{ci}")
                nc.gpsimd.iota(pii[:], pattern=[[0, QC], [1, 4]], base=0,
                               channel_multiplier=0)
                nc.scalar.copy(out=PIDc[:], in_=pii[:])
                Lc = st.tile([P, QC], f32, tag=f"L{ci}")
                nc.gpsimd.memset(Lc[:], 0.0)
                W.append(Wc); PID.append(PIDc); L.append(Lc)

            for step in range(1, DEPTH + 1):
                for ci in range(NCHUNK):
                    Wc, PIDc, Lc = W[ci], PID[ci], L[ci]
                    W3 = Wc[:].rearrange("p (q s) -> p q s", s=4)

                    Li = tp.tile([P, QC], i32, tag=f"Li{ci}")
                    nc.scalar.copy(out=Li[:], in_=Lc[:])
                    R = gp.tile([P, QC * 6], f32, tag=f"R{ci}")
                    # one offset per partition per instruction (the only
                    # HW-reliable vector-DGE configuration)
                    for qi in range(QC):
                        nc.gpsimd.indirect_dma_start(
                            out=R[:, qi * 6:(qi + 1) * 6], out_offset=None,
                            in_=tabs[:],
                            in_offset=bass.IndirectOffsetOnAxis(
                                ap=Li[:, qi:qi + 1], axis=0),
                            element_offset=0)
                    R3 = R[:].rearrange("p (q s) -> p q s", s=6)
                    Dv = R3[:, :, 0:4]
                    p0v = R3[:, :, 4]
                    pdv = R3[:, :, 5]

                    t = tp.tile([P, QC * 4], f32, tag=f"t{ci}")
                    t3 = t[:].rearrange("p (q s) -> p q s", s=4)
                    nc.vector.tensor_tensor(out=t3, in0=Dv, in1=W3,
                                            op=AL.mult)
                    dd = tp.tile([P, QC], f32, tag=f"dd{ci}")
                    nc.vector.tensor_reduce(out=dd[:], in_=t3,
                                            axis=mybir.AxisListType.X,
                                            op=AL.add)
                    u = tp.tile([P, QC * 4], f32, tag=f"u{ci}")
                    u3 = u[:].rearrange("p (q s) -> p q s", s=4)
                    nc.vector.tensor_tensor(out=u3, in0=t3, in1=Dv,
                                            op=AL.mult)
                    sw = tp.tile([P, QC], f32, tag=f"sw{ci}")
                    nc.vector.tensor_reduce(out=sw[:], in_=u3,
                                            axis=mybir.AxisListType.X,
                                            op=AL.add)
                    g = tp.tile([P, QC], f32, tag=f"g{ci}")
                    nc.vector.tensor_scalar(out=g[:], in0=dd[:], scalar1=0.0,
                                            scalar2=None, op0=AL.is_gt)
                    gh = tp.tile([P, QC], f32, tag=f"gh{ci}")
                    nc.vector.tensor_scalar(out=gh[:], in0=dd[:], scalar1=0.0,
                                            scalar2=-0.5, op0=AL.is_gt,
                                            op1=AL.add)
                    q1 = tp.tile([P, QC], f32, tag=f"q1{ci}")
                    nc.vector.tensor_tensor(out=q1[:], in0=gh[:], in1=dd[:],
                                            op=AL.mult)
                    wmax2 = tp.tile([P, QC * 4], f32, tag=f"wmax2{ci}")
                    nc.vector.scalar_tensor_tensor(
                        out=wmax2[:], in0=q1[:].broadcast_to([P, QC, 4]),
                        scalar=2.0, in1=sw[:].broadcast_to([P, QC, 4]),
                        op0=AL.mult, op1=AL.add)
                    wkm = tp.tile([P, QC * 4], f32, tag=f"wkm{ci}")
                    nc.vector.scalar_tensor_tensor(
                        out=wkm[:], in0=dd[:].broadcast_to([P, QC, 4]),
                        scalar=-1.0, in1=dd[:].broadcast_to([P, QC, 4]),
                        op0=AL.mult, op1=AL.min)
                    S = tp.tile([P, QC * 4], i8, tag=f"S{ci}")
                    S3 = S[:].rearrange("p (q s) -> p q s", s=4)
                    nc.vector.tensor_scalar(out=S3, in0=Dv, scalar1=0.0,
                                            scalar2=None, op0=AL.not_equal)
                    ghB = gh[:].broadcast_to([P, QC, 4])
                    Dgh = tp.tile([P, QC * 4], f32, tag=f"Dgh{ci}")
                    Dgh3 = Dgh[:].rearrange("p (q s) -> p q s", s=4)
                    nc.vector.tensor_tensor(out=Dgh3, in0=Dv, in1=ghB,
                                            op=AL.mult)
                    A = tp.tile([P, QC * 4], i8, tag=f"A{ci}")
                    A3 = A[:].rearrange("p (q s) -> p q s", s=4)
                    nc.vector.tensor_scalar(out=A3, in0=Dgh3, scalar1=0.0,
                                            scalar2=None, op0=AL.is_gt)
                    nc.vector.copy_predicated(out=Wc[:], mask=S[:],
                                              data=wkm[:])
                    nc.vector.copy_predicated(out=Wc[:], mask=A[:],
                                              data=wmax2[:])
                    pg = tp.tile([P, QC * 4], f32, tag=f"pg{ci}")
                    nc.gpsimd.tensor_tensor(
                        out=pg[:], in0=g[:].broadcast_to([P, QC, 4]),
                        in1=pdv.broadcast_to([P, QC, 4]), op=AL.mult)
                    psel = tp.tile([P, QC * 4], f32, tag=f"psel{ci}")
                    nc.gpsimd.tensor_tensor(
                        out=psel[:], in0=pg[:],
                        in1=p0v.broadcast_to([P, QC, 4]), op=AL.add)
                    nc.vector.copy_predicated(out=PIDc[:], mask=A[:],
                                              data=psel[:])
                    g1 = tp.tile([P, QC], f32, tag=f"g1{ci}")
                    nc.vector.tensor_scalar(out=g1[:], in0=dd[:], scalar1=0.0,
                                            scalar2=1.0, op0=AL.is_gt,
                                            op1=AL.add)
                    nc.vector.scalar_tensor_tensor(
                        out=Lc[:], in0=Lc[:], scalar=2.0, in1=g1[:],
                        op0=AL.mult, op1=AL.add)

            for ci in range(NCHUNK):
                Wc, PIDc = W[ci], PID[ci]
                pidI = tp.tile([P, QC * 4], i32, tag=f"pidI{ci}")
                nc.scalar.copy(out=pidI[:], in_=PIDc[:])
                for s in range(FSUB):
                    FG = gp.tile([P, QF * 4 * F], f32, tag="FG")
                    for qi in range(QF * 4):
                        col = s * QF * 4 + qi
                        nc.gpsimd.indirect_dma_start(
                            out=FG[:, qi * F:(qi + 1) * F], out_offset=None,
                            in_=field[:],
                            in_offset=bass.IndirectOffsetOnAxis(
                                ap=pidI[:, col:col + 1], axis=0),
                            element_offset=0)
                    F4 = FG[:].rearrange("p (q s f) -> p q s f", s=4, f=F)
                    w4 = Wc[:].rearrange("p (q s) -> p q s", s=4)[
                        :, s * QF:(s + 1) * QF, :]
                    wB = w4.rearrange("p q s -> p (q s)").rearrange(
                        "p (q s o) -> p q s o", s=4, o=1).broadcast_to(
                        [P, QF, 4, F])
                    y = gp.tile([P, QF * 4 * F], f32, tag="y")
                    y4 = y[:].rearrange("p (q s f) -> p q s f", s=4, f=F)
                    nc.vector.tensor_tensor(out=y4, in0=F4, in1=wB,
                                            op=AL.mult)
                    z1 = tp.tile([P, QF * 2 * F], f32, tag="z1")
                    z14 = z1[:].rearrange("p (q s f) -> p q s f", s=2, f=F)
                    nc.gpsimd.tensor_tensor(out=z14, in0=y4[:, :, 0:2, :],
                                            in1=y4[:, :, 2:4, :], op=AL.add)
                    z = tp.tile([P, QF * F], f32, tag="z")
                    z3 = z[:].rearrange("p (q f) -> p q f", f=F)
                    nc.vector.tensor_tensor(out=z3, in0=z14[:, :, 0, :],
                                            in1=z14[:, :, 1, :], op=AL.add)
                    qlo = ci * QC + s * QF
                    nc.sync.dma_start(
                        out=outv[:, qlo * F:(qlo + QF) * F], in_=z[:])
    return nc


@functools.lru_cache(maxsize=1)
def _compiled_kernel(minv_key):
    minv = np.frombuffer(minv_key, dtype=np.float32).reshape(4, 4)
    nc = bacc.Bacc("TRN2", target_bir_lowering=False, debug=False,
                   num_devices=N_CORES)
    _build_kernel(nc, minv)
    nc.compile()
    return nc


def kernel(xyz, field, root_xyz, child_index, point_index, child_cut,
           activation_layer):
    xyz = np.asarray(xyz, dtype=np.float32)
    field = np.asarray(field, dtype=np.float32)
    root_xyz = np.asarray(root_xyz, dtype=np.float32)
    child_cut = np.asarray(child_cut)
    point_index = np.asarray(point_index)

    tabs = _build_tables(child_cut, point_index)
    minv = _minv_from_root(root_xyz)
    nc = _compiled_kernel(minv.tobytes())

    NQ_CORE = P * QP
    in_maps = []
    for k in range(N_CORES):
        xs = xyz[k * NQ_CORE:(k + 1) * NQ_CORE]
        in_maps.append({
            "xyzf": np.ascontiguousarray(xs.reshape(P, QP * 3)),
            "tabs": tabs,
            "field": field,
        })
    res = run_bass_kernel_spmd(nc, in_maps, list(range(N_CORES)))
    return np.concatenate(
        [res.results[k]["out"] for k in range(N_CORES)], axis=0)


# revision 4
# speedup vs baseline: 1.6511x; 1.6511x over previous
"""MultiLayerTetra TRN2 Bass kernel (8-core SPMD, data-parallel over queries).

Algorithm: the reference's per-step batched 4x4 solve collapses to an
incremental barycentric update. Per descent step, with cut pair (c0,c1) of
the current cell and barycentric weights w:
    d = w[c0] - w[c1]; choice g = [d > 0]
    abandoned a = cut slot with larger w, kept k = the other
    w[k] <- w[k] - w[a]  (= -|d|),  w[a] <- 2*w[a]  (= sum + |d|)
    pid[a] <- point_index[child, a];  cell <- 2*cell + 1 + g
Final: out = sum_j w_j * field[pid_j].  (Validated vs reference: ~1e-6 rel.)

Per-cell table row (6 f32): D = onehot(c0) - onehot(c1), p0, p1 - p0 where
p0/p1 are the pids injected when descending with choice 0/1.

Device mapping per core (16384 queries): query (p, q) at partition p, free
slot q; slot-interleaved W/PID state [128, QP*4]. Gathers use the [P,1]
per-partition indirect-DMA config (one offset per partition per
instruction), which is the HW-reliable vector-DGE shape.
"""
import functools
import numpy as np

import concourse.bass as bass
import concourse.bacc as bacc
import concourse.mybir as mybir
from concourse.tile import TileContext
from concourse.bass_utils import run_bass_kernel_spmd

DEPTH = 18
NT = 2 ** DEPTH - 1
P = 128
F = 32
N_CORES = 8
QP = 128
NCHUNK = 2
FIELD_ROWS = 100000

AL = mybir.AluOpType
AF = mybir.ActivationFunctionType


def _cell_cols(child_cut, point_index, cells):
    # per-cell step data: D = onehot(c0)-onehot(c1), p0, p1-p0
    cut0 = child_cut[cells, 0].astype(np.int64)
    cut1 = child_cut[cells, 1].astype(np.int64)
    eye = np.eye(4, dtype=np.float32)
    D = eye[cut0] - eye[cut1]
    p0 = point_index[2 * cells + 1, cut1].astype(np.float32)
    p1 = point_index[2 * cells + 2, cut0].astype(np.float32)
    return D, p0, p1 - p0


def _build_tables(child_cut, point_index):
    # 2-step rows (18 f32): own-step data for cell c plus both children's
    # step data (child selected on-chip by the first step's choice).
    c = np.arange(NT)
    D, p0, pd = _cell_cols(child_cut, point_index, c)
    # child-step columns: only needed for even-level cells (gathers happen
    # at odd steps); level-17 cells' children are leaves, so zero-fill.
    ci_ = np.arange(2 ** (DEPTH - 1) - 1)
    Da = np.zeros((NT, 4), np.float32); p0a = np.zeros(NT, np.float32)
    pda = np.zeros(NT, np.float32)
    Db = np.zeros((NT, 4), np.float32); p0b = np.zeros(NT, np.float32)
    pdb = np.zeros(NT, np.float32)
    da, pa0, pad = _cell_cols(child_cut, point_index, 2 * ci_ + 1)
    db, pb0, pbd = _cell_cols(child_cut, point_index, 2 * ci_ + 2)
    Da[ci_], p0a[ci_], pda[ci_] = da, pa0, pad
    Db[ci_], p0b[ci_], pdb[ci_] = db, pb0, pbd
    cols = [D, p0[:, None], pd[:, None], Da, Db - Da,
            p0a[:, None], pda[:, None],
            (p0b - p0a)[:, None], (pdb - pda)[:, None]]
    return np.concatenate(cols, axis=1).astype(np.float32)


def _minv_from_root(root_xyz):
    M = np.concatenate(
        [root_xyz.T.astype(np.float64), np.ones((1, 4), np.float64)], axis=0)
    return np.linalg.inv(M).astype(np.float32)


def _build_kernel(nc, minv):
    f32 = mybir.dt.float32
    i32 = mybir.dt.int32
    i8 = mybir.dt.int8
    NQ = P * QP
    QC = QP // NCHUNK
    FSUB = max(1, QC // 32)
    QF = QC // FSUB

    xyzf = nc.dram_tensor("xyzf", [P, QP * 3], f32, kind="ExternalInput")
    tabs = nc.dram_tensor("tabs", [NT, 18], f32, kind="ExternalInput")
    field = nc.dram_tensor("field", [FIELD_ROWS, F], f32,
                           kind="ExternalInput")
    out = nc.dram_tensor("out", [NQ, F], f32, kind="ExternalOutput")
    outv = out[:].rearrange("(p q) f -> p (q f)", p=P)

    with TileContext(nc) as tc:
        with tc.tile_pool(name="state", bufs=1) as st, \
             tc.tile_pool(name="tmp", bufs=2) as tp, \
             tc.tile_pool(name="gath", bufs=2) as gp:

            xyzs = st.tile([P, QP * 3], f32, tag="xyzs")
            nc.sync.dma_start(out=xyzs[:], in_=xyzf[:])
            xyz3 = xyzs[:].rearrange("p (q c) -> p q c", c=3)

            W, PID, L = [], [], []
            for ci in range(NCHUNK):
                qlo = ci * QC
                Xv = xyz3[:, qlo:qlo + QC, 0]
                Yv = xyz3[:, qlo:qlo + QC, 1]
                Zv = xyz3[:, qlo:qlo + QC, 2]
                Wc = st.tile([P, QC * 4], f32, tag=f"W{ci}")
                W3 = Wc[:].rearrange("p (q s) -> p q s", s=4)
                for j in range(4):
                    a1 = tp.tile([P, QC], f32, tag=f"ia1_{ci}")
                    nc.scalar.activation(a1[:], Zv, AF.Copy,
                                         bias=float(minv[j, 3]),
                                         scale=float(minv[j, 2]))
                    a2 = tp.tile([P, QC], f32, tag=f"ia2_{ci}")
                    nc.vector.scalar_tensor_tensor(
                        out=a2[:], in0=Yv, scalar=float(minv[j, 1]),
                        in1=a1[:], op0=AL.mult, op1=AL.add)
                    nc.vector.scalar_tensor_tensor(
                        out=W3[:, :, j], in0=Xv, scalar=float(minv[j, 0]),
                        in1=a2[:], op0=AL.mult, op1=AL.add)
                PIDc = st.tile([P, QC * 4], f32, tag=f"PID{ci}")
                pii = tp.tile([P, QC * 4], i32, tag=f"pii{ci}")
                nc.gpsimd.iota(pii[:], pattern=[[0, QC], [1, 4]], base=0,
                               channel_multiplier=0)
                nc.scalar.copy(out=PIDc[:], in_=pii[:])
                Lc = st.tile([P, QC], f32, tag=f"L{ci}")
                nc.gpsimd.memset(Lc[:], 0.0)
                W.append(Wc); PID.append(PIDc); L.append(Lc)

            def step_body(ci, Dv, p0v, pdv):
                Wc, PIDc, Lc = W[ci], PID[ci], L[ci]
                W3 = Wc[:].rearrange("p (q s) -> p q s", s=4)
                t = tp.tile([P, QC * 4], f32, tag=f"t{ci}")
                t3 = t[:].rearrange("p (q s) -> p q s", s=4)
                nc.vector.tensor_tensor(out=t3, in0=Dv, in1=W3, op=AL.mult)
                dd = tp.tile([P, QC], f32, tag=f"dd{ci}")
                nc.vector.tensor_reduce(out=dd[:], in_=t3,
                                        axis=mybir.AxisListType.X, op=AL.add)
                u = tp.tile([P, QC * 4], f32, tag=f"u{ci}")
                u3 = u[:].rearrange("p (q s) -> p q s", s=4)
                nc.vector.tensor_tensor(out=u3, in0=t3, in1=Dv, op=AL.mult)
                sw = tp.tile([P, QC], f32, tag=f"sw{ci}")
                nc.vector.tensor_reduce(out=sw[:], in_=u3,
                                        axis=mybir.AxisListType.X, op=AL.add)
                g = tp.tile([P, QC], f32, tag=f"g{ci}")
                nc.vector.tensor_scalar(out=g[:], in0=dd[:], scalar1=0.0,
                                        scalar2=None, op0=AL.is_gt)
                gh = tp.tile([P, QC], f32, tag=f"gh{ci}")
                nc.vector.tensor_scalar(out=gh[:], in0=dd[:], scalar1=0.0,
                                        scalar2=-0.5, op0=AL.is_gt,
                                        op1=AL.add)
                q1 = tp.tile([P, QC], f32, tag=f"q1{ci}")
                nc.vector.tensor_tensor(out=q1[:], in0=gh[:], in1=dd[:],
                                        op=AL.mult)
                wmax2 = tp.tile([P, QC * 4], f32, tag=f"wmax2{ci}")
                nc.vector.scalar_tensor_tensor(
                    out=wmax2[:], in0=q1[:].broadcast_to([P, QC, 4]),
                    scalar=2.0, in1=sw[:].broadcast_to([P, QC, 4]),
                    op0=AL.mult, op1=AL.add)
                wkm = tp.tile([P, QC * 4], f32, tag=f"wkm{ci}")
                nc.vector.scalar_tensor_tensor(
                    out=wkm[:], in0=dd[:].broadcast_to([P, QC, 4]),
                    scalar=-1.0, in1=dd[:].broadcast_to([P, QC, 4]),
                    op0=AL.mult, op1=AL.min)
                S = tp.tile([P, QC * 4], i8, tag=f"S{ci}")
                S3 = S[:].rearrange("p (q s) -> p q s", s=4)
                nc.vector.tensor_scalar(out=S3, in0=Dv, scalar1=0.0,
                                        scalar2=None, op0=AL.not_equal)
                ghB = gh[:].broadcast_to([P, QC, 4])
                Dgh = tp.tile([P, QC * 4], f32, tag=f"Dgh{ci}")
                Dgh3 = Dgh[:].rearrange("p (q s) -> p q s", s=4)
                nc.vector.tensor_tensor(out=Dgh3, in0=Dv, in1=ghB,
                                        op=AL.mult)
                A = tp.tile([P, QC * 4], i8, tag=f"A{ci}")
                A3 = A[:].rearrange("p (q s) -> p q s", s=4)
                nc.vector.tensor_scalar(out=A3, in0=Dgh3, scalar1=0.0,
                                        scalar2=None, op0=AL.is_gt)
                nc.vector.copy_predicated(out=Wc[:], mask=S[:], data=wkm[:])
                nc.vector.copy_predicated(out=Wc[:], mask=A[:],
                                          data=wmax2[:])
                pg = tp.tile([P, QC * 4], f32, tag=f"pg{ci}")
                nc.gpsimd.tensor_tensor(
                    out=pg[:], in0=g[:].broadcast_to([P, QC, 4]),
                    in1=pdv.broadcast_to([P, QC, 4]), op=AL.mult)
                psel = tp.tile([P, QC * 4], f32, tag=f"psel{ci}")
                nc.gpsimd.tensor_tensor(
                    out=psel[:], in0=pg[:],
                    in1=p0v.broadcast_to([P, QC, 4]), op=AL.add)
                nc.vector.copy_predicated(out=PIDc[:], mask=A[:],
                                          data=psel[:])
                g1 = tp.tile([P, QC], f32, tag=f"g1{ci}")
                nc.vector.tensor_scalar(out=g1[:], in0=dd[:], scalar1=0.0,
                                        scalar2=1.0, op0=AL.is_gt,
                                        op1=AL.add)
                nc.vector.scalar_tensor_tensor(
                    out=Lc[:], in0=Lc[:], scalar=2.0, in1=g1[:],
                    op0=AL.mult, op1=AL.add)
                return g

            for step in range(1, DEPTH + 1, 2):
                for ci in range(NCHUNK):
                    Lc = L[ci]
                    Li = tp.tile([P, QC], i32, tag=f"Li{ci}")
                    nc.scalar.copy(out=Li[:], in_=Lc[:])
                    R = gp.tile([P, QC * 18], f32, tag=f"R{ci}")
                    # one offset per partition per instruction (the only
                    # HW-reliable vector-DGE configuration); each 72B row
                    # carries this step's data plus both children's.
                    for qi in range(QC):
                        nc.gpsimd.indirect_dma_start(
                            out=R[:, qi * 18:(qi + 1) * 18], out_offset=None,
                            in_=tabs[:],
                            in_offset=bass.IndirectOffsetOnAxis(
                                ap=Li[:, qi:qi + 1], axis=0),
                            element_offset=0)
                    R3 = R[:].rearrange("p (q s) -> p q s", s=18)
                    g = step_body(ci, R3[:, :, 0:4], R3[:, :, 4],
                                  R3[:, :, 5])
                    # select the chosen child's step data: x = a + g*(b - a)
                    gB = g[:].broadcast_to([P, QC, 4])
                    D2 = tp.tile([P, QC * 4], f32, tag=f"D2{ci}")
                    D23 = D2[:].rearrange("p (q s) -> p q s", s=4)
                    nc.vector.tensor_tensor(out=D23, in0=gB,
                                            in1=R3[:, :, 10:14], op=AL.mult)
                    nc.vector.tensor_tensor(out=D23, in0=D23,
                                            in1=R3[:, :, 6:10], op=AL.add)
                    pp2 = tp.tile([P, QC * 2], f32, tag=f"pp2{ci}")
                    pp23 = pp2[:].rearrange("p (q s) -> p q s", s=2)
                    nc.gpsimd.tensor_tensor(
                        out=pp23, in0=g[:].broadcast_to([P, QC, 2]),
                        in1=R3[:, :, 16:18], op=AL.mult)
                    nc.gpsimd.tensor_tensor(out=pp23, in0=pp23,
                                            in1=R3[:, :, 14:16], op=AL.add)
                    step_body(ci, D23, pp23[:, :, 0], pp23[:, :, 1])

            for ci in range(NCHUNK):
                Wc, PIDc = W[ci], PID[ci]
                pidI = tp.tile([P, QC * 4], i32, tag=f"pidI{ci}")
                nc.scalar.copy(out=pidI[:], in_=PIDc[:])
                for s in range(FSUB):
                    FG = gp.tile([P, QF * 4 * F], f32, tag="FG")
                    for qi in range(QF * 4):
                        col = s * QF * 4 + qi
                        nc.gpsimd.indirect_dma_start(
                            out=FG[:, qi * F:(qi + 1) * F], out_offset=None,
                            in_=field[:],
                            in_offset=bass.IndirectOffsetOnAxis(
                                ap=pidI[:, col:col + 1], axis=0),
                            element_offset=0)
                    F4 = FG[:].rearrange("p (q s f) -> p q s f", s=4, f=F)
                    w4 = Wc[:].rearrange("p (q s) -> p q s", s=4)[
                        :, s * QF:(s + 1) * QF, :]
                    wB = w4.rearrange("p q s -> p (q s)").rearrange(
                        "p (q s o) -> p q s o", s=4, o=1).broadcast_to(
                        [P, QF, 4, F])
                    y = gp.tile([P, QF * 4 * F], f32, tag="y")
                    y4 = y[:].rearrange("p (q s f) -> p q s f", s=4, f=F)
                    nc.vector.tensor_tensor(out=y4, in0=F4, in1=wB,
                                            op=AL.mult)
                    z1 = tp.tile([P, QF * 2 * F], f32, tag="z1")
                    z14 = z1[:].rearrange("p (q s f) -> p q s f", s=2, f=F)
                    nc.gpsimd.tensor_tensor(out=z14, in0=y4[:, :, 0:2, :],
                                            in1=y4[:, :, 2:4, :], op=AL.add)
                    z = tp.tile([P, QF * F], f32, tag="z")
                    z3 = z[:].rearrange("p (q f) -> p q f", f=F)
                    nc.vector.tensor_tensor(out=z3, in0=z14[:, :, 0, :],
                                            in1=z14[:, :, 1, :], op=AL.add)
                    qlo = ci * QC + s * QF
                    nc.sync.dma_start(
                        out=outv[:, qlo * F:(qlo + QF) * F], in_=z[:])
    return nc


@functools.lru_cache(maxsize=1)
def _compiled_kernel(minv_key):
    minv = np.frombuffer(minv_key, dtype=np.float32).reshape(4, 4)
    nc = bacc.Bacc("TRN2", target_bir_lowering=False, debug=False,
                   num_devices=N_CORES)
    _build_kernel(nc, minv)
    nc.compile()
    return nc


def kernel(xyz, field, root_xyz, child_index, point_index, child_cut,
           activation_layer):
    xyz = np.asarray(xyz, dtype=np.float32)
    field = np.asarray(field, dtype=np.float32)
    root_xyz = np.asarray(root_xyz, dtype=np.float32)
    child_cut = np.asarray(child_cut)
    point_index = np.asarray(point_index)

    tabs = _build_tables(child_cut, point_index)
    minv = _minv_from_root(root_xyz)
    nc = _compiled_kernel(minv.tobytes())

    NQ_CORE = P * QP
    in_maps = []
    for k in range(N_CORES):
        xs = xyz[k * NQ_CORE:(k + 1) * NQ_CORE]
        in_maps.append({
            "xyzf": np.ascontiguousarray(xs.reshape(P, QP * 3)),
            "tabs": tabs,
            "field": field,
        })
    res = run_bass_kernel_spmd(nc, in_maps, list(range(N_CORES)))
    return np.concatenate(
        [res.results[k]["out"] for k in range(N_CORES)], axis=0)


# revision 5
# speedup vs baseline: 1.7692x; 1.0715x over previous
"""MultiLayerTetra TRN2 Bass kernel (8-core SPMD, data-parallel over queries).

Algorithm: the reference's per-step batched 4x4 solve collapses to an
incremental barycentric update. Per descent step, with cut pair (c0,c1) of
the current cell and barycentric weights w:
    d = w[c0] - w[c1]; choice g = [d > 0]
    abandoned a = cut slot with larger w, kept k = the other
    w[k] <- w[k] - w[a]  (= -|d|),  w[a] <- 2*w[a]  (= sum + |d|)
    pid[a] <- point_index[child, a];  cell <- 2*cell + 1 + g
Final: out = sum_j w_j * field[pid_j].  (Validated vs reference: ~1e-6 rel.)

Per-cell table row (6 f32): D = onehot(c0) - onehot(c1), p0, p1 - p0 where
p0/p1 are the pids injected when descending with choice 0/1.

Device mapping per core (16384 queries): query (p, q) at partition p, free
slot q; slot-interleaved W/PID state [128, QP*4]. Gathers use the [P,1]
per-partition indirect-DMA config (one offset per partition per
instruction), which is the HW-reliable vector-DGE shape.
"""
import functools
import numpy as np

import concourse.bass as bass
import concourse.bacc as bacc
import concourse.mybir as mybir
from concourse.tile import TileContext
from concourse.bass_utils import run_bass_kernel_spmd

DEPTH = 18
NT = 2 ** DEPTH - 1
P = 128
F = 32
N_CORES = 8
QP = 128
NCHUNK = 2
FIELD_ROWS = 100000

AL = mybir.AluOpType
AF = mybir.ActivationFunctionType


def _cell_cols(child_cut, point_index, cells):
    # per-cell step data: D = onehot(c0)-onehot(c1), p0, p1-p0
    cut0 = child_cut[cells, 0].astype(np.int64)
    cut1 = child_cut[cells, 1].astype(np.int64)
    eye = np.eye(4, dtype=np.float32)
    D = eye[cut0] - eye[cut1]
    p0 = point_index[2 * cells + 1, cut1].astype(np.float32)
    p1 = point_index[2 * cells + 2, cut0].astype(np.float32)
    return D, p0, p1 - p0


def _build_tables(child_cut, point_index):
    # 2-step rows (18 f32): own-step data for cell c plus both children's
    # step data (child selected on-chip by the first step's choice).
    c = np.arange(NT)
    D, p0, pd = _cell_cols(child_cut, point_index, c)
    # child-step columns: only needed for even-level cells (gathers happen
    # at odd steps); level-17 cells' children are leaves, so zero-fill.
    ci_ = np.arange(2 ** (DEPTH - 1) - 1)
    Da = np.zeros((NT, 4), np.float32); p0a = np.zeros(NT, np.float32)
    pda = np.zeros(NT, np.float32)
    Db = np.zeros((NT, 4), np.float32); p0b = np.zeros(NT, np.float32)
    pdb = np.zeros(NT, np.float32)
    da, pa0, pad = _cell_cols(child_cut, point_index, 2 * ci_ + 1)
    db, pb0, pbd = _cell_cols(child_cut, point_index, 2 * ci_ + 2)
    Da[ci_], p0a[ci_], pda[ci_] = da, pa0, pad
    Db[ci_], p0b[ci_], pdb[ci_] = db, pb0, pbd
    cols = [D, p0[:, None], pd[:, None], Da, Db - Da,
            p0a[:, None], pda[:, None],
            (p0b - p0a)[:, None], (pdb - pda)[:, None]]
    return np.concatenate(cols, axis=1).astype(np.float32)


def _minv_from_root(root_xyz):
    M = np.concatenate(
        [root_xyz.T.astype(np.float64), np.ones((1, 4), np.float64)], axis=0)
    return np.linalg.inv(M).astype(np.float32)


def _build_kernel(nc, minv):
    f32 = mybir.dt.float32
    i32 = mybir.dt.int32
    i8 = mybir.dt.int8
    NQ = P * QP
    QC = QP // NCHUNK
    FSUB = max(1, QC // 32)
    QF = QC // FSUB

    xyzf = nc.dram_tensor("xyzf", [P, QP * 3], f32, kind="ExternalInput")
    tabs = nc.dram_tensor("tabs", [NT, 18], f32, kind="ExternalInput")
    field = nc.dram_tensor("field", [FIELD_ROWS, F], f32,
                           kind="ExternalInput")
    out = nc.dram_tensor("out", [NQ, F], f32, kind="ExternalOutput")
    outv = out[:].rearrange("(p q) f -> p (q f)", p=P)

    with TileContext(nc) as tc:
        with tc.tile_pool(name="state", bufs=1) as st, \
             tc.tile_pool(name="tmp", bufs=2) as tp, \
             tc.tile_pool(name="gath", bufs=2) as gp:

            xyzs = st.tile([P, QP * 3], f32, tag="xyzs")
            nc.sync.dma_start(out=xyzs[:], in_=xyzf[:])
            xyz3 = xyzs[:].rearrange("p (q c) -> p q c", c=3)

            W, PID, L = [], [], []
            for ci in range(NCHUNK):
                qlo = ci * QC
                Xv = xyz3[:, qlo:qlo + QC, 0]
                Yv = xyz3[:, qlo:qlo + QC, 1]
                Zv = xyz3[:, qlo:qlo + QC, 2]
                Wc = st.tile([P, QC * 4], f32, tag=f"W{ci}")
                W3 = Wc[:].rearrange("p (q s) -> p q s", s=4)
                for j in range(4):
                    a1 = tp.tile([P, QC], f32, tag=f"ia1_{ci}")
                    nc.scalar.activation(a1[:], Zv, AF.Copy,
                                         bias=float(minv[j, 3]),
                                         scale=float(minv[j, 2]))
                    a2 = tp.tile([P, QC], f32, tag=f"ia2_{ci}")
                    nc.vector.scalar_tensor_tensor(
                        out=a2[:], in0=Yv, scalar=float(minv[j, 1]),
                        in1=a1[:], op0=AL.mult, op1=AL.add)
                    nc.vector.scalar_tensor_tensor(
                        out=W3[:, :, j], in0=Xv, scalar=float(minv[j, 0]),
                        in1=a2[:], op0=AL.mult, op1=AL.add)
                PIDc = st.tile([P, QC * 4], f32, tag=f"PID{ci}")
                pii = tp.tile([P, QC * 4], i32, tag=f"pii{ci}")
                nc.gpsimd.iota(pii[:], pattern=[[0, QC], [1, 4]], base=0,
                               channel_multiplier=0)
                nc.scalar.copy(out=PIDc[:], in_=pii[:])
                Lc = st.tile([P, QC], f32, tag=f"L{ci}")
                nc.gpsimd.memset(Lc[:], 0.0)
                W.append(Wc); PID.append(PIDc); L.append(Lc)

            def step_body(ci, Dv, p0v, pdv):
                Wc, PIDc, Lc = W[ci], PID[ci], L[ci]
                W3 = Wc[:].rearrange("p (q s) -> p q s", s=4)
                t = tp.tile([P, QC * 4], f32, tag=f"t{ci}")
                t3 = t[:].rearrange("p (q s) -> p q s", s=4)
                nc.vector.tensor_tensor(out=t3, in0=Dv, in1=W3, op=AL.mult)
                dd = tp.tile([P, QC], f32, tag=f"dd{ci}")
                nc.vector.tensor_reduce(out=dd[:], in_=t3,
                                        axis=mybir.AxisListType.X, op=AL.add)
                u = tp.tile([P, QC * 4], f32, tag=f"u{ci}")
                u3 = u[:].rearrange("p (q s) -> p q s", s=4)
                nc.vector.tensor_tensor(out=u3, in0=t3, in1=Dv, op=AL.mult)
                sw = tp.tile([P, QC], f32, tag=f"sw{ci}")
                nc.vector.tensor_reduce(out=sw[:], in_=u3,
                                        axis=mybir.AxisListType.X, op=AL.add)
                g = tp.tile([P, QC], f32, tag=f"g{ci}")
                nc.vector.tensor_scalar(out=g[:], in0=dd[:], scalar1=0.0,
                                        scalar2=None, op0=AL.is_gt)
                gh = tp.tile([P, QC], f32, tag=f"gh{ci}")
                nc.vector.tensor_scalar(out=gh[:], in0=dd[:], scalar1=0.0,
                                        scalar2=-0.5, op0=AL.is_gt,
                                        op1=AL.add)
                q1 = tp.tile([P, QC], f32, tag=f"q1{ci}")
                nc.vector.tensor_tensor(out=q1[:], in0=gh[:], in1=dd[:],
                                        op=AL.mult)
                wmax2 = tp.tile([P, QC * 4], f32, tag=f"wmax2{ci}")
                nc.vector.scalar_tensor_tensor(
                    out=wmax2[:], in0=q1[:].broadcast_to([P, QC, 4]),
                    scalar=2.0, in1=sw[:].broadcast_to([P, QC, 4]),
                    op0=AL.mult, op1=AL.add)
                wkm = tp.tile([P, QC * 4], f32, tag=f"wkm{ci}")
                nc.vector.scalar_tensor_tensor(
                    out=wkm[:], in0=dd[:].broadcast_to([P, QC, 4]),
                    scalar=-1.0, in1=dd[:].broadcast_to([P, QC, 4]),
                    op0=AL.mult, op1=AL.min)
                S = tp.tile([P, QC * 4], i8, tag=f"S{ci}")
                S3 = S[:].rearrange("p (q s) -> p q s", s=4)
                nc.vector.tensor_scalar(out=S3, in0=Dv, scalar1=0.0,
                                        scalar2=None, op0=AL.not_equal)
                ghB = gh[:].broadcast_to([P, QC, 4])
                Dgh = tp.tile([P, QC * 4], f32, tag=f"Dgh{ci}")
                Dgh3 = Dgh[:].rearrange("p (q s) -> p q s", s=4)
                nc.vector.tensor_tensor(out=Dgh3, in0=Dv, in1=ghB,
                                        op=AL.mult)
                A = tp.tile([P, QC * 4], i8, tag=f"A{ci}")
                A3 = A[:].rearrange("p (q s) -> p q s", s=4)
                nc.vector.tensor_scalar(out=A3, in0=Dgh3, scalar1=0.0,
                                        scalar2=None, op0=AL.is_gt)
                nc.vector.copy_predicated(out=Wc[:], mask=S[:], data=wkm[:])
                nc.vector.copy_predicated(out=Wc[:], mask=A[:],
                                          data=wmax2[:])
                pg = tp.tile([P, QC * 4], f32, tag=f"pg{ci}")
                nc.gpsimd.tensor_tensor(
                    out=pg[:], in0=g[:].broadcast_to([P, QC, 4]),
                    in1=pdv.broadcast_to([P, QC, 4]), op=AL.mult)
                psel = tp.tile([P, QC * 4], f32, tag=f"psel{ci}")
                nc.gpsimd.tensor_tensor(
                    out=psel[:], in0=pg[:],
                    in1=p0v.broadcast_to([P, QC, 4]), op=AL.add)
                nc.vector.copy_predicated(out=PIDc[:], mask=A[:],
                                          data=psel[:])
                g1 = tp.tile([P, QC], f32, tag=f"g1{ci}")
                nc.vector.tensor_scalar(out=g1[:], in0=dd[:], scalar1=0.0,
                                        scalar2=1.0, op0=AL.is_gt,
                                        op1=AL.add)
                nc.vector.scalar_tensor_tensor(
                    out=Lc[:], in0=Lc[:], scalar=2.0, in1=g1[:],
                    op0=AL.mult, op1=AL.add)
                return g

            for step in range(1, DEPTH + 1, 2):
                for ci in range(NCHUNK):
                    Lc = L[ci]
                    Li = tp.tile([P, QC], i32, tag=f"Li{ci}")
                    nc.scalar.copy(out=Li[:], in_=Lc[:])
                    R = gp.tile([P, QC * 18], f32, tag=f"R{ci}")
                    if step == 1:
                        # every query is at the root: gather row 0 once and
                        # broadcast it across all query slots
                        nc.gpsimd.indirect_dma_start(
                            out=R[:, 0:18], out_offset=None, in_=tabs[:],
                            in_offset=bass.IndirectOffsetOnAxis(
                                ap=Li[:, 0:1], axis=0),
                            element_offset=0)
                        RT = R[:].rearrange("p (q s) -> p s q", s=18)
                        nc.vector.tensor_copy(
                            out=RT[:, :, 1:QC],
                            in_=R[:, 0:18].broadcast_to([P, 18, QC - 1]))
                    else:
                        # one offset per partition per instruction (the only
                        # HW-reliable vector-DGE configuration); each 72B row
                        # carries this step's data plus both children's.
                        for qi in range(QC):
                            nc.gpsimd.indirect_dma_start(
                                out=R[:, qi * 18:(qi + 1) * 18],
                                out_offset=None, in_=tabs[:],
                                in_offset=bass.IndirectOffsetOnAxis(
                                    ap=Li[:, qi:qi + 1], axis=0),
                                element_offset=0)
                    R3 = R[:].rearrange("p (q s) -> p q s", s=18)
                    g = step_body(ci, R3[:, :, 0:4], R3[:, :, 4],
                                  R3[:, :, 5])
                    # select the chosen child's step data: x = a + g*(b - a)
                    gB = g[:].broadcast_to([P, QC, 4])
                    D2 = tp.tile([P, QC * 4], f32, tag=f"D2{ci}")
                    D23 = D2[:].rearrange("p (q s) -> p q s", s=4)
                    nc.vector.tensor_tensor(out=D23, in0=gB,
                                            in1=R3[:, :, 10:14], op=AL.mult)
                    nc.vector.tensor_tensor(out=D23, in0=D23,
                                            in1=R3[:, :, 6:10], op=AL.add)
                    pp2 = tp.tile([P, QC * 2], f32, tag=f"pp2{ci}")
                    pp23 = pp2[:].rearrange("p (q s) -> p q s", s=2)
                    nc.gpsimd.tensor_tensor(
                        out=pp23, in0=g[:].broadcast_to([P, QC, 2]),
                        in1=R3[:, :, 16:18], op=AL.mult)
                    nc.gpsimd.tensor_tensor(out=pp23, in0=pp23,
                                            in1=R3[:, :, 14:16], op=AL.add)
                    step_body(ci, D23, pp23[:, :, 0], pp23[:, :, 1])

            for ci in range(NCHUNK):
                Wc, PIDc = W[ci], PID[ci]
                pidI = tp.tile([P, QC * 4], i32, tag=f"pidI{ci}")
                nc.scalar.copy(out=pidI[:], in_=PIDc[:])
                for s in range(FSUB):
                    FG = gp.tile([P, QF * 4 * F], f32, tag="FG")
                    for qi in range(QF * 4):
                        col = s * QF * 4 + qi
                        nc.gpsimd.indirect_dma_start(
                            out=FG[:, qi * F:(qi + 1) * F], out_offset=None,
                            in_=field[:],
                            in_offset=bass.IndirectOffsetOnAxis(
                                ap=pidI[:, col:col + 1], axis=0),
                            element_offset=0)
                    F4 = FG[:].rearrange("p (q s f) -> p q s f", s=4, f=F)
                    w4 = Wc[:].rearrange("p (q s) -> p q s", s=4)[
                        :, s * QF:(s + 1) * QF, :]
                    wB = w4.rearrange("p q s -> p (q s)").rearrange(
                        "p (q s o) -> p q s o", s=4, o=1).broadcast_to(
                        [P, QF, 4, F])
                    y = gp.tile([P, QF * 4 * F], f32, tag="y")
                    y4 = y[:].rearrange("p (q s f) -> p q s f", s=4, f=F)
                    nc.vector.tensor_tensor(out=y4, in0=F4, in1=wB,
                                            op=AL.mult)
                    z1 = tp.tile([P, QF * 2 * F], f32, tag="z1")
                    z14 = z1[:].rearrange("p (q s f) -> p q s f", s=2, f=F)
                    nc.gpsimd.tensor_tensor(out=z14, in0=y4[:, :, 0:2, :],
                                            in1=y4[:, :, 2:4, :], op=AL.add)
                    z = tp.tile([P, QF * F], f32, tag="z")
                    z3 = z[:].rearrange("p (q f) -> p q f", f=F)
                    nc.vector.tensor_tensor(out=z3, in0=z14[:, :, 0, :],
                                            in1=z14[:, :, 1, :], op=AL.add)
                    qlo = ci * QC + s * QF
                    nc.sync.dma_start(
                        out=outv[:, qlo * F:(qlo + QF) * F], in_=z[:])
    return nc


@functools.lru_cache(maxsize=1)
def _compiled_kernel(minv_key):
    minv = np.frombuffer(minv_key, dtype=np.float32).reshape(4, 4)
    nc = bacc.Bacc("TRN2", target_bir_lowering=False, debug=False,
                   num_devices=N_CORES)
    _build_kernel(nc, minv)
    nc.compile()
    return nc


def kernel(xyz, field, root_xyz, child_index, point_index, child_cut,
           activation_layer):
    xyz = np.asarray(xyz, dtype=np.float32)
    field = np.asarray(field, dtype=np.float32)
    root_xyz = np.asarray(root_xyz, dtype=np.float32)
    child_cut = np.asarray(child_cut)
    point_index = np.asarray(point_index)

    tabs = _build_tables(child_cut, point_index)
    minv = _minv_from_root(root_xyz)
    nc = _compiled_kernel(minv.tobytes())

    NQ_CORE = P * QP
    in_maps = []
    for k in range(N_CORES):
        xs = xyz[k * NQ_CORE:(k + 1) * NQ_CORE]
        in_maps.append({
            "xyzf": np.ascontiguousarray(xs.reshape(P, QP * 3)),
            "tabs": tabs,
            "field": field,
        })
    res = run_bass_kernel_spmd(nc, in_maps, list(range(N_CORES)))
    return np.concatenate(
        [res.results[k]["out"] for k in range(N_CORES)], axis=0)
